# revision 2
# baseline (speedup 1.0000x reference)
"""Trainium2 Bass kernel for BinaryLinearUnit:
    y = sign(x) @ sign(w).T ; BatchNorm1d(train) ; * gamma + beta

Strategy: data-parallel over the batch dim across 8 NeuronCores.
Each core computes y.T for its 1024-row batch slice with an FP8
(DoubleRow) matmul on the tensor engine — sign values are exactly
representable in fp8e4m3, and PSUM accumulates in fp32, so the matmul
is exact. Per-core BN partial stats (mean, E[y^2] per channel) are
combined with an 8-core AllReduce, then each core normalizes its slice
and writes y.T back.

Host side only reshapes/transposes: x and w are fed K-major (the
contraction dim IN must sit on SBUF partitions for the PE), the
per-core output arrives as y.T and is transposed back.
"""

import numpy as np

import concourse.bass as bass
import concourse.mybir as mybir
import concourse.tile as tile
from concourse import bacc
from concourse.bass import ts
from concourse.bass_utils import run_bass_kernel_spmd

N_CORES = 8
BN_EPS = 1e-5

f32 = mybir.dt.float32
f16 = mybir.dt.float16
fp8 = mybir.dt.float8e4


def build(B, IN, OUT, n_cores=N_CORES, use_fp8=True):
    """Build the per-core SPMD module. Shapes: x [B, IN], w [OUT, IN]."""
    Bc = B // n_cores          # batch rows per core
    KT = IN // 128             # k tiles (contraction)
    OT = OUT // 128            # output-feature tiles
    NB = min(512, Bc)          # matmul free dim / psum bank width
    BT = Bc // NB              # b tiles per core
    XH = min(512, Bc)          # x load/sign chunk width
    XHT = Bc // XH

    act_dt = fp8 if use_fp8 else mybir.dt.bfloat16

    nc = bacc.Bacc("TRN2", target_bir_lowering=False, debug=False,
                   num_devices=n_cores)

    # Per-core external I/O (host pre-transposed, K-major):
    #   xt[k, b] = x[core*Bc + b, k]
    #   w2[ot, p, ks, o] = w[ot*128 + o, ks*128 + p]
    #   yt[o, b] = out[core*Bc + b, o]
    xt = nc.dram_tensor("xt", [IN, Bc], f32, kind="ExternalInput")
    w2 = nc.dram_tensor("w2", [OT, 128, KT, 128], f32, kind="ExternalInput")
    gamma = nc.dram_tensor("gamma", [OUT], f32, kind="ExternalInput")
    beta = nc.dram_tensor("beta", [OUT], f32, kind="ExternalInput")
    yt = nc.dram_tensor("yt", [OUT, Bc], f32, kind="ExternalOutput")

    # Collective bounce buffers (per-channel [mean/8, E[y^2]/8])
    ccin = nc.dram_tensor("ccin", [128, 2 * OT], f32)
    ccout = nc.dram_tensor("ccout", [128, 2 * OT], f32, addr_space="Shared")

    with tile.TileContext(nc) as tc:
        with (
            tc.tile_pool(name="big", bufs=1) as big,
            tc.tile_pool(name="xs", bufs=3) as xsp,
            tc.tile_pool(name="ws", bufs=2) as wsp,
            tc.tile_pool(name="sw", bufs=2) as swp,
            tc.tile_pool(name="ps", bufs=2, space="PSUM") as psp,
            tc.tile_pool(name="st", bufs=4) as stp,
            tc.tile_pool(name="outp", bufs=4) as outp,
        ):
            # Standing tensors
            sxT = big.tile([128, KT, Bc], act_dt)       # sign(x).T, K-major
            yTt = big.tile([128, OT, Bc], f16)          # y.T (exact in fp16)
            mvT = big.tile([128, 2, OT], f32)           # per-core [mean, var]
            gam = big.tile([128, OT], f32)
            bet = big.tile([128, OT], f32)

            # ---- load x, compute sign(x) into sxT (bt-half 0 first so the
            # PE can start before the second half arrives) ----
            for xh in range(XHT):
                for ks in range(KT):
                    xst = xsp.tile([128, XH], f32, tag="xst")
                    nc.sync.dma_start(
                        out=xst[:],
                        in_=xt[ts(ks, 128), ts(xh, XH)],
                    )
                    nc.scalar.sign(sxT[:, ks, ts(xh, XH)], xst[:])

            nc.sync.dma_start(out=gam[:], in_=gamma[:].rearrange("(t p) -> p t", p=128))
            nc.sync.dma_start(out=bet[:], in_=beta[:].rearrange("(t p) -> p t", p=128))

            # ---- main loop over output-feature tiles ----
            for ot in range(OT):
                wst = wsp.tile([128, KT, 128], f32, tag="wst")
                nc.sync.dma_start(out=wst[:], in_=w2[ot, :, :, :])
                swt = swp.tile([128, KT, 128], act_dt, tag="swt")
                nc.scalar.sign(swt[:], wst[:])

                psums = [
                    psp.tile([128, NB], f32, tag=f"ps{bt}", name=f"psum{bt}")
                    for bt in range(BT)
                ]
                if use_fp8:
                    KP = KT // 2
                    for kp in range(KP):
                        for bt in range(BT):
                            nc.tensor.matmul(
                                psums[bt][:],
                                lhsT=swt[:, 2 * kp : 2 * kp + 2, :],
                                rhs=sxT[:, 2 * kp : 2 * kp + 2, ts(bt, NB)],
                                start=(kp == 0),
                                stop=(kp == KP - 1),
                                perf_mode=mybir.MatmulPerfMode.DoubleRow,
                            )
                else:
                    for k in range(KT):
                        for bt in range(BT):
                            nc.tensor.matmul(
                                psums[bt][:],
                                lhsT=swt[:, k, :],
                                rhs=sxT[:, k, ts(bt, NB)],
                                start=(k == 0),
                                stop=(k == KT - 1),
                            )

                st6 = stp.tile([128, BT, 6], f32, tag="st6")
                for bt in range(BT):
                    nc.vector.bn_stats(st6[:, bt, :], psums[bt][:])
                    nc.scalar.copy(yTt[:, ot, ts(bt, NB)], psums[bt][:])
                nc.vector.bn_aggr(mvT[:, :, ot], st6[:])

            # ---- global BN stats via AllReduce of [mean/8, E[y^2]/8] ----
            arT = big.tile([128, 2, OT], f32)
            tmp = big.tile([128, OT], f32)
            nc.vector.tensor_scalar_mul(arT[:, 0, :], mvT[:, 0, :], 1.0 / n_cores)
            nc.vector.tensor_mul(tmp[:], mvT[:, 0, :], mvT[:, 0, :])
            nc.vector.tensor_add(tmp[:], tmp[:], mvT[:, 1, :])
            nc.vector.tensor_scalar_mul(arT[:, 1, :], tmp[:], 1.0 / n_cores)
            nc.sync.dma_start(out=ccin[:], in_=arT[:])
            nc.gpsimd.collective_compute(
                "AllReduce",
                mybir.AluOpType.add,
                replica_groups=[list(range(n_cores))],
                ins=[ccin[:]],
                outs=[ccout[:]],
            )
            grT = big.tile([128, 2, OT], f32)
            nc.sync.dma_start(out=grT[:], in_=ccout[:])

            # var = E[y^2] - mean^2 ; rstd = rsqrt(var + eps) (Newton-refined)
            gmean = grT[:, 0, :]
            gvar = big.tile([128, OT], f32)
            veps = big.tile([128, OT], f32)
            nc.vector.tensor_mul(gvar[:], gmean, gmean)
            nc.vector.tensor_sub(gvar[:], grT[:, 1, :], gvar[:])
            nc.vector.tensor_scalar_add(veps[:], gvar[:], BN_EPS)
            sq = big.tile([128, OT], f32)
            nc.scalar.sqrt(sq[:], veps[:])
            r = big.tile([128, OT], f32)
            nc.vector.reciprocal(r[:], sq[:])
            t2 = big.tile([128, OT], f32)
            for _ in range(2):  # Newton: r <- r * (1.5 - 0.5 * veps * r^2)
                nc.vector.tensor_mul(t2[:], veps[:], r[:])
                nc.vector.tensor_mul(t2[:], t2[:], r[:])
                nc.vector.tensor_scalar(t2[:], t2[:], -0.5, 1.5,
                                        op0=mybir.AluOpType.mult,
                                        op1=mybir.AluOpType.add)
                nc.vector.tensor_mul(r[:], r[:], t2[:])

            scal = big.tile([128, OT], f32)
            nbias = big.tile([128, OT], f32)
            nc.vector.tensor_mul(scal[:], gam[:], r[:])
            nc.vector.tensor_mul(t2[:], gmean, scal[:])
            nc.vector.tensor_sub(nbias[:], bet[:], t2[:])

            # ---- normalize and store ----
            for ot in range(OT):
                for bt in range(BT):
                    ob = outp.tile([128, NB], f32, tag="ob")
                    nc.vector.tensor_scalar(
                        ob[:],
                        yTt[:, ot, ts(bt, NB)],
                        scal[:, ot : ot + 1],
                        nbias[:, ot : ot + 1],
                        op0=mybir.AluOpType.mult,
                        op1=mybir.AluOpType.add,
                    )
                    nc.sync.dma_start(
                        out=yt[ts(ot, 128), ts(bt, NB)], in_=ob[:]
                    )

    nc.finalize()
    return nc


def shard_inputs(x, w, gamma, beta, n_cores=N_CORES):
    B, IN = x.shape
    OUT = w.shape[0]
    Bc = B // n_cores
    KT, OT = IN // 128, OUT // 128
    w2 = np.ascontiguousarray(
        w.reshape(OT, 128, KT, 128).transpose(0, 3, 2, 1)
    )
    in_maps = []
    for c in range(n_cores):
        xt = np.ascontiguousarray(x[c * Bc : (c + 1) * Bc].T)
        in_maps.append(
            {"xt": xt, "w2": w2, "gamma": gamma, "beta": beta}
        )
    return in_maps


_NC_CACHE = {}


def kernel(x, w, gamma, beta):
    x = np.asarray(x)
    w = np.asarray(w)
    gamma = np.asarray(gamma)
    beta = np.asarray(beta)
    B, IN = x.shape
    OUT = w.shape[0]

    key = (B, IN, OUT)
    if key not in _NC_CACHE:
        _NC_CACHE[key] = build(B, IN, OUT)
    nc = _NC_CACHE[key]

    in_maps = shard_inputs(x, w, gamma, beta)
    res = run_bass_kernel_spmd(nc, in_maps, list(range(N_CORES)))
    out = np.concatenate([r["yt"] for r in res.results], axis=1).T
    return np.ascontiguousarray(out)


if __name__ == "__main__":
    rng = np.random.default_rng(0)
    B, IN, OUT = 8192, 4096, 4096
    x = rng.standard_normal((B, IN)).astype(np.float32)
    w = rng.standard_normal((OUT, IN)).astype(np.float32)
    gamma = np.ones(OUT, np.float32)
    beta = np.zeros(OUT, np.float32)
    out = kernel(x, w, gamma, beta)
    print(out.shape, out.dtype)


# revision 4
# speedup vs baseline: 1.1277x; 1.1277x over previous
"""Trainium2 Bass kernel for BinaryLinearUnit:
    y = sign(x) @ sign(w).T ; BatchNorm1d(train) ; * gamma + beta

Strategy: data-parallel over the batch dim across 8 NeuronCores.
Each core computes y.T for its 1024-row batch slice with an FP8
(DoubleRow) matmul on the tensor engine — sign values are exactly
representable in fp8e4m3, and PSUM accumulates in fp32, so the matmul
is exact. Per-core BN partial stats (mean, E[y^2] per channel) are
combined with an 8-core AllReduce, then each core normalizes its slice
and writes y.T back. The stats/AllReduce/normalize pipeline is split
into two halves over the output features so the first half overlaps
the second half's matmuls.

Host side only reshapes/transposes: x and w are fed K-major (the
contraction dim IN must sit on SBUF partitions for the PE), the
per-core output arrives as y.T and is transposed back.
"""

import numpy as np

import concourse.bass as bass
import concourse.mybir as mybir
import concourse.tile as tile
from concourse import bacc
from concourse.bass import ts
from concourse.bass_utils import run_bass_kernel_spmd

N_CORES = 8
BN_EPS = 1e-5

f32 = mybir.dt.float32
f16 = mybir.dt.float16
fp8 = mybir.dt.float8e4


def build(B, IN, OUT, n_cores=N_CORES, use_fp8=True):
    """Build the per-core SPMD module. Shapes: x [B, IN], w [OUT, IN]."""
    Bc = B // n_cores          # batch rows per core
    KT = IN // 128             # k tiles (contraction)
    OT = OUT // 128            # output-feature tiles
    NB = min(512, Bc)          # matmul free dim / psum bank width
    BT = Bc // NB              # b tiles per core
    XH = min(512, Bc)          # x load/sign chunk width
    XHT = Bc // XH
    assert OT % 2 == 0
    HOT = OT // 2              # output tiles per stats half

    act_dt = fp8 if use_fp8 else mybir.dt.bfloat16

    nc = bacc.Bacc("TRN2", target_bir_lowering=False, debug=False,
                   num_devices=n_cores)

    # Per-core external I/O (host pre-transposed, K-major):
    #   xt[k, b] = x[core*Bc + b, k]
    #   w2[ot, p, ks, o] = w[ot*128 + o, ks*128 + p]
    #   yt[o, b] = out[core*Bc + b, o]
    xt = nc.dram_tensor("xt", [IN, Bc], f32, kind="ExternalInput")
    w2 = nc.dram_tensor("w2", [OT, 128, KT, 128], f32, kind="ExternalInput")
    gamma = nc.dram_tensor("gamma", [OUT], f32, kind="ExternalInput")
    beta = nc.dram_tensor("beta", [OUT], f32, kind="ExternalInput")
    yt = nc.dram_tensor("yt", [OUT, Bc], f32, kind="ExternalOutput")

    # Collective bounce buffers per half: [mean/8, E[y^2]/8] per channel
    ccin = [
        nc.dram_tensor(f"ccin{h}", [128, 2 * HOT], f32) for h in range(2)
    ]
    ccout = [
        nc.dram_tensor(f"ccout{h}", [128, 2 * HOT], f32, addr_space="Shared")
        for h in range(2)
    ]

    with tile.TileContext(nc) as tc:
        with (
            tc.tile_pool(name="big", bufs=1) as big,
            tc.tile_pool(name="xs", bufs=3) as xsp,
            tc.tile_pool(name="ws", bufs=3) as wsp,
            tc.tile_pool(name="sw", bufs=3) as swp,
            tc.tile_pool(name="ps", bufs=2, space="PSUM") as psp,
            tc.tile_pool(name="st", bufs=4) as stp,
            tc.tile_pool(name="outp", bufs=4) as outp,
        ):
            # Standing tensors
            sxT = big.tile([128, KT, Bc], act_dt)       # sign(x).T, K-major
            yTt = big.tile([128, OT, Bc], f16)          # y.T (exact in fp16)
            mvT = big.tile([128, 2, OT], f32)           # per-core [mean, var]
            gam = big.tile([128, OT], f32)
            bet = big.tile([128, OT], f32)
            scal = big.tile([128, OT], f32)             # gamma * rstd
            nbias = big.tile([128, OT], f32)            # beta - mean * scal

            def w_chain(ot):
                wst = wsp.tile([128, KT, 128], f32, tag="wst", name="wst")
                nc.sync.dma_start(out=wst[:], in_=w2[ot, :, :, :])
                swt = swp.tile([128, KT, 128], act_dt, tag="swt", name="swt")
                nc.scalar.sign(swt[:], wst[:])
                return swt

            def mm_tile(ot, swt):
                psums = [
                    psp.tile([128, NB], f32, tag=f"ps{bt}", name=f"psum{bt}")
                    for bt in range(BT)
                ]
                if use_fp8:
                    KP = KT // 2
                    for kp in range(KP):
                        for bt in range(BT):
                            nc.tensor.matmul(
                                psums[bt][:],
                                lhsT=swt[:, 2 * kp : 2 * kp + 2, :],
                                rhs=sxT[:, 2 * kp : 2 * kp + 2, ts(bt, NB)],
                                start=(kp == 0),
                                stop=(kp == KP - 1),
                                perf_mode=mybir.MatmulPerfMode.DoubleRow,
                            )
                else:
                    for k in range(KT):
                        for bt in range(BT):
                            nc.tensor.matmul(
                                psums[bt][:],
                                lhsT=swt[:, k, :],
                                rhs=sxT[:, k, ts(bt, NB)],
                                start=(k == 0),
                                stop=(k == KT - 1),
                            )
                st6 = stp.tile([128, BT, 6], f32, tag="st6", name="st6")
                for bt in range(BT):
                    nc.vector.bn_stats(st6[:, bt, :], psums[bt][:])
                    nc.vector.tensor_copy(yTt[:, ot, ts(bt, NB)], psums[bt][:])
                nc.vector.bn_aggr(mvT[:, :, ot], st6[:])

            def stats_half(h):
                """Combine this half's per-core stats into global scale/bias."""
                o0 = h * HOT
                osl = slice(o0, o0 + HOT)
                arT = stp.tile([128, 2, HOT], f32, tag="arT", name="arT", bufs=2)
                tmp = stp.tile([128, HOT], f32, tag="tmp_ar", name="tmp_ar", bufs=2)
                nc.vector.tensor_scalar_mul(arT[:, 0, :], mvT[:, 0, osl], 1.0 / n_cores)
                nc.vector.tensor_mul(tmp[:], mvT[:, 0, osl], mvT[:, 0, osl])
                nc.vector.tensor_add(tmp[:], tmp[:], mvT[:, 1, osl])
                nc.vector.tensor_scalar_mul(arT[:, 1, :], tmp[:], 1.0 / n_cores)
                nc.sync.dma_start(out=ccin[h][:], in_=arT[:])
                nc.gpsimd.collective_compute(
                    "AllReduce",
                    mybir.AluOpType.add,
                    replica_groups=[list(range(n_cores))],
                    ins=[ccin[h][:]],
                    outs=[ccout[h][:]],
                )
                grT = stp.tile([128, 2, HOT], f32, tag="grT", name="grT", bufs=2)
                nc.sync.dma_start(out=grT[:], in_=ccout[h][:])

                # var = E[y^2] - mean^2 ; rstd = rsqrt(var+eps), Newton-refined
                gmean = grT[:, 0, :]
                gvar = stp.tile([128, HOT], f32, tag="gvar", name="gvar", bufs=2)
                veps = stp.tile([128, HOT], f32, tag="veps", name="veps", bufs=2)
                nc.vector.tensor_mul(gvar[:], gmean, gmean)
                nc.vector.tensor_sub(gvar[:], grT[:, 1, :], gvar[:])
                nc.vector.tensor_scalar_add(veps[:], gvar[:], BN_EPS)
                sq = stp.tile([128, HOT], f32, tag="sq", name="sq", bufs=2)
                nc.scalar.sqrt(sq[:], veps[:])
                r = stp.tile([128, HOT], f32, tag="r", name="rstd", bufs=2)
                nc.vector.reciprocal(r[:], sq[:])
                t2 = stp.tile([128, HOT], f32, tag="t2", name="t2", bufs=2)
                for _ in range(2):  # Newton: r <- r * (1.5 - 0.5 * veps * r^2)
                    nc.vector.tensor_mul(t2[:], veps[:], r[:])
                    nc.vector.tensor_mul(t2[:], t2[:], r[:])
                    nc.vector.tensor_scalar(t2[:], t2[:], -0.5, 1.5,
                                            op0=mybir.AluOpType.mult,
                                            op1=mybir.AluOpType.add)
                    nc.vector.tensor_mul(r[:], r[:], t2[:])
                nc.vector.tensor_mul(scal[:, osl], gam[:, osl], r[:])
                nc.vector.tensor_mul(t2[:], gmean, scal[:, osl])
                nc.vector.tensor_sub(nbias[:, osl], bet[:, osl], t2[:])

            def norm_half(h):
                for ot in range(h * HOT, (h + 1) * HOT):
                    for bt in range(BT):
                        ob = outp.tile([128, NB], f32, tag="ob", name="ob")
                        nc.vector.tensor_scalar(
                            ob[:],
                            yTt[:, ot, ts(bt, NB)],
                            scal[:, ot : ot + 1],
                            nbias[:, ot : ot + 1],
                            op0=mybir.AluOpType.mult,
                            op1=mybir.AluOpType.add,
                        )
                        nc.sync.dma_start(
                            out=yt[ts(ot, 128), ts(bt, NB)], in_=ob[:]
                        )

            # ---- emission order == scheduling priority ----
            # ot=0 weight chain first so the PE can start ASAP
            swt0 = w_chain(0)

            # x sign: bt-half 0 first (first matmuls only need that half)
            for xh in range(XHT):
                for ks in range(KT):
                    xst = xsp.tile([128, XH], f32, tag="xst", name="xst")
                    nc.sync.dma_start(
                        out=xst[:], in_=xt[ts(ks, 128), ts(xh, XH)]
                    )
                    nc.scalar.sign(sxT[:, ks, ts(xh, XH)], xst[:])

            nc.sync.dma_start(out=gam[:], in_=gamma[:].rearrange("(t p) -> p t", p=128))
            nc.sync.dma_start(out=bet[:], in_=beta[:].rearrange("(t p) -> p t", p=128))

            # first half of the output features
            swt_next = None
            for ot in range(HOT):
                swt = swt0 if ot == 0 else swt_next
                if ot + 1 < OT:
                    swt_next = w_chain(ot + 1)
                mm_tile(ot, swt)
            stats_half(0)
            # second half (its matmuls overlap half 0's AllReduce + norm)
            for ot in range(HOT, OT):
                swt = swt_next
                if ot + 1 < OT:
                    swt_next = w_chain(ot + 1)
                mm_tile(ot, swt)
            stats_half(1)
            norm_half(0)
            norm_half(1)

    nc.finalize()
    return nc


def shard_inputs(x, w, gamma, beta, n_cores=N_CORES):
    B, IN = x.shape
    OUT = w.shape[0]
    Bc = B // n_cores
    KT, OT = IN // 128, OUT // 128
    w2 = np.ascontiguousarray(
        w.reshape(OT, 128, KT, 128).transpose(0, 3, 2, 1)
    )
    in_maps = []
    for c in range(n_cores):
        xt = np.ascontiguousarray(x[c * Bc : (c + 1) * Bc].T)
        in_maps.append(
            {"xt": xt, "w2": w2, "gamma": gamma, "beta": beta}
        )
    return in_maps


_NC_CACHE = {}


def kernel(x, w, gamma, beta):
    x = np.asarray(x)
    w = np.asarray(w)
    gamma = np.asarray(gamma)
    beta = np.asarray(beta)
    B, IN = x.shape
    OUT = w.shape[0]

    key = (B, IN, OUT)
    if key not in _NC_CACHE:
        _NC_CACHE[key] = build(B, IN, OUT)
    nc = _NC_CACHE[key]

    in_maps = shard_inputs(x, w, gamma, beta)
    res = run_bass_kernel_spmd(nc, in_maps, list(range(N_CORES)))
    out = np.concatenate([r["yt"] for r in res.results], axis=1).T
    return np.ascontiguousarray(out)


if __name__ == "__main__":
    rng = np.random.default_rng(0)
    B, IN, OUT = 8192, 4096, 4096
    x = rng.standard_normal((B, IN)).astype(np.float32)
    w = rng.standard_normal((OUT, IN)).astype(np.float32)
    gamma = np.ones(OUT, np.float32)
    beta = np.zeros(OUT, np.float32)
    out = kernel(x, w, gamma, beta)
    print(out.shape, out.dtype)
